# revision 16
# baseline (speedup 1.0000x reference)
"""Fused cross-attention kernel for Trainium2 (Bass/Tile), 8-core SPMD.

Problem: query/key_value [T=4, B=2, C=128, H=32, W=32] -> tokens [B, N=4096, C],
QKV projections (128x128), full softmax attention over N tokens per batch.

Sharding: core = b*4 + t handles batch b, query tokens [t*1024, (t+1)*1024)
against all 4096 K/V tokens of batch b.

Algebraic restructure (vs. materializing Q/K/V):
  scores:  S^T[m,n] = x_kv[m] . qk[n]   with  qk = (Wk^T Wq) x_q + Wk^T bq
           (A = Wk^T Wq precomputed on host; bk shifts all scores of a row
           equally and drops out of softmax exactly)
  output:  O^T = Wv Z / rowsum,  Z[c,n] = sum_m x_kv[m,c] P[m,n]
           (V-projection pulled out of the attention sum by linearity)
So the device only runs: one 128x128 projection (qk), the two big
attention matmuls (S and Z), one final 128x128 matmul (Wv Z), exp, and
16-bit rowsum accumulation. No K/V projection matmuls, no per-chunk
PSUM->SBUF projection copies.

Per m-chunk (128 kv tokens, 32 chunks):
  pss [m=128, n=1024] = kvx_chunk^T @ qk       (PE, bf16, 2x512-col matmuls)
  p   = exp(SCALE * pss)                       (ACT, PSUM->SBUF, 16-bit out)
  psz [c=128, n=1024] += kvxT_chunk^T @ p      (PE, accumulated over chunks)
  acc_i += p                                   (DVE 2-byte 2x-mode adds)
Rowsums land pre-transposed via tiny PE matmuls acc_i^T @ ones accumulated
in PSUM [n-part, nb]; normalization and the final [C,NQ]->[NQ,C] transpose
happen on host (host already assembles shards and adds bv).

ldweights-only filler instructions (no PSUM write, no semaphores) can be
interleaved to keep the PE busy streak alive for the HAM clock ramp.

Inputs prepacked bf16 on host; P is 16-bit (exp <= e^7.7 ~ 2200 fits both
f16/bf16; validated ~3.6e-3 rel err end-to-end vs the 2e-2 gate).
"""

import math
from contextlib import ExitStack

import numpy as np
import ml_dtypes

import concourse.bass as bass
import concourse.mybir as mybir
import concourse.tile as tile
from concourse import bacc
from concourse.bass_utils import run_bass_kernel_spmd

F32 = mybir.dt.float32
F32R = mybir.dt.float32r
BF16 = mybir.dt.bfloat16
F16 = mybir.dt.float16
AF = mybir.ActivationFunctionType

C = 128        # model dim
NQ = 1024      # query tokens per core
M = 4096       # kv tokens per batch
NCH = M // 128 # m chunks
T = 4
B = 2
SCALE = 1.0 / math.sqrt(float(C))
N_CORES = 8

CFG = dict(
    p_dtype="f16",  # "f16" | "bf16" | "f32r": exp output / kvxT / rowsum dtype
    p_bufs=8,       # exp output SBUF buffers
    ps_s_bufs=3,    # score PSUM buffers ([128,1024] = 2 banks each)
    pe_warm=24,     # ldweights warm-ups during the DMA window
    fillers=2,      # ldweights fillers per chunk (hold the PE HAM streak)
    head_fill=8,    # ldweights fillers between qk proj and chunk 0
    z_on_act=True,  # drain Z psum->sbuf on ACT (idle after last exp)
    fillers_dep=True,  # fillers read p (un-hoistable, interleave per chunk)
    unroll=4,       # kernel bodies per For_i iteration (amortizes barrier)
)

_P_DT = {"f16": F16, "bf16": BF16, "f32r": F32R}
_P_NP = {"f16": np.float16, "bf16": ml_dtypes.bfloat16, "f32r": np.float32}
_N_CHAINS = {"f16": 2, "bf16": 4, "f32r": 2}

_NC = None


def build_nc(reps=1, loop_reps=0, **overrides):
    cfg = dict(CFG)
    cfg.update(overrides)
    p_dt = _P_DT[cfg["p_dtype"]]
    acc_dt = F32 if cfg["p_dtype"] == "f32r" else p_dt
    n_chains = _N_CHAINS[cfg["p_dtype"]]

    nc = bacc.Bacc()
    qx = nc.dram_tensor("qx", [C, NQ], BF16, kind="ExternalInput")
    aT = nc.dram_tensor("aT", [C, C], BF16, kind="ExternalInput")
    bqk = nc.dram_tensor("bqk", [C, 1], F32, kind="ExternalInput")
    wvT = nc.dram_tensor("wvT", [C, C], BF16, kind="ExternalInput")
    kvx = nc.dram_tensor("kvx", [C, M], BF16, kind="ExternalInput")
    kvxT = nc.dram_tensor("kvxT", [C, M], p_dt, kind="ExternalInput")
    out2 = nc.dram_tensor("out2", [C, NQ + (NQ // C) * n_chains], F32,
                          kind="ExternalOutput")

    unroll = cfg["unroll"]
    if loop_reps and loop_reps % unroll == 0 and loop_reps >= unroll:
        loop_iters, reps = loop_reps // unroll, reps * unroll
    elif loop_reps:
        loop_iters = loop_reps
    else:
        loop_iters = 0

    with tile.TileContext(nc) as tc, ExitStack() as ctx:
        const = ctx.enter_context(tc.tile_pool(name="const", bufs=1))
        proj = ctx.enter_context(tc.tile_pool(name="proj", bufs=1))
        pwork = ctx.enter_context(tc.tile_pool(name="pwork", bufs=cfg["p_bufs"]))
        owork = ctx.enter_context(tc.tile_pool(name="owork", bufs=2))
        psum = ctx.enter_context(tc.tile_pool(name="psum", bufs=2, space="PSUM"))

        def misc_tile(name):
            # borrow a rotating score-PSUM buffer for small/late matmuls
            return psum.tile([128, NQ], F32, tag="ps_s",
                             bufs=cfg["ps_s_bufs"], name=name)

        # Constants (gpsimd, no DMA deps). Warm the exp table first so the
        # 1.5us table load overlaps the input DMAs / NEFF preamble.
        ones_f32 = const.tile([128, 1], F32)
        nc.gpsimd.memset(ones_f32, 1.0)
        warm = const.tile([128, 1], F32)
        nc.scalar.activation(warm, ones_f32, AF.Exp)
        ones_p = const.tile([128, 1], acc_dt)
        nc.gpsimd.memset(ones_p, 1.0)
        warm_w = const.tile([128, 128], BF16)
        nc.gpsimd.memset(warm_w, 1.0)

        # PE warm-up: ldweights-only ops (no PSUM, no cross-engine deps)
        # to build a continuous-busy streak for the HAM clock ramp.
        for _w in range(cfg["pe_warm"]):
            nc.tensor.ldweights(warm_w)

        # Input DMAs, spread across the SP and ACT HWDGE rings.
        qx_sb = const.tile([C, NQ], BF16)
        nc.sync.dma_start(qx_sb, qx[:])
        aT_sb = const.tile([C, C], BF16)
        nc.sync.dma_start(aT_sb, aT[:])
        bqk_sb = const.tile([C, 1], F32)
        nc.sync.dma_start(bqk_sb, bqk[:])
        wvT_sb = const.tile([C, C], BF16)
        nc.sync.dma_start(wvT_sb, wvT[:])
        kvx_sb = const.tile([C, M], BF16)
        nc.sync.dma_start(kvx_sb[:, 0:1024], kvx[:, 0:1024])
        nc.scalar.dma_start(kvx_sb[:, 1024:2560], kvx[:, 1024:2560])
        nc.scalar.dma_start(kvx_sb[:, 2560:4096], kvx[:, 2560:4096])
        kvxT_sb = const.tile([C, M], p_dt)
        nc.sync.dma_start(kvxT_sb[:, 0:2048], kvxT[:, 0:2048])
        nc.scalar.dma_start(kvxT_sb[:, 2048:4096], kvxT[:, 2048:4096])

        loop_cm = tc.For_i(0, loop_iters, 1) if loop_iters else None
        if loop_cm is not None:
            loop_cm.__enter__()
        for _rep in range(reps):
            # ---- qk projection: qk = A @ qx + bqk ----
            # (matmul outputs may not cross a PSUM bank: 512 f32 cols max)
            psq = misc_tile("psq")
            for h in range(2):
                nc.tensor.matmul(psq[:, h * 512:(h + 1) * 512], lhsT=aT_sb,
                                 rhs=qx_sb[:, h * 512:(h + 1) * 512],
                                 start=True, stop=True)
            qk_sb = proj.tile([C, NQ], BF16, tag="qk_sb", bufs=2,
                              name="qk_sb")
            with nc.allow_low_precision(reason="bf16 qk tokens"):
                for h in range(2):
                    nc.vector.tensor_scalar_add(
                        qk_sb[:, h * 512:(h + 1) * 512],
                        psq[:, h * 512:(h + 1) * 512], bqk_sb)
            for _f in range(cfg["head_fill"]):
                nc.tensor.ldweights(warm_w)

            # ---- chunk loop ----
            psz = psum.tile([128, NQ], F32, tag="ps_z", bufs=1, name="psz")
            accs = [owork.tile([128, NQ], acc_dt, tag=f"acc{i}", bufs=1,
                               name=f"acc{i}") for i in range(n_chains)]
            for j in range(NCH):
                pss = psum.tile([128, NQ], F32, tag="ps_s",
                                bufs=cfg["ps_s_bufs"])
                for h in range(2):
                    nc.tensor.matmul(pss[:, h * 512:(h + 1) * 512],
                                     lhsT=kvx_sb[:, j * 128:(j + 1) * 128],
                                     rhs=qk_sb[:, h * 512:(h + 1) * 512],
                                     start=True, stop=True)
                if cfg["fillers"] and not cfg["fillers_dep"]:
                    for _f in range((cfg["fillers"] + 1) // 2):
                        nc.tensor.ldweights(warm_w)
                p = pwork.tile([128, NQ], p_dt, tag="p_sb", bufs=cfg["p_bufs"])
                nc.scalar.activation(p, pss, AF.Exp, scale=SCALE)
                for h in range(2):
                    nc.tensor.matmul(psz[:, h * 512:(h + 1) * 512],
                                     lhsT=kvxT_sb[:, j * 128:(j + 1) * 128],
                                     rhs=p[:, h * 512:(h + 1) * 512],
                                     start=(j == 0), stop=(j == NCH - 1))
                if cfg["fillers"]:
                    nf = (cfg["fillers"] if cfg["fillers_dep"]
                          else cfg["fillers"] // 2)
                    for _f in range(nf):
                        # reading p makes the filler depend on this chunk's
                        # exp, so the scheduler cannot hoist it into a blob
                        nc.tensor.ldweights(p[:, _f * 128:(_f + 1) * 128])
                acc = accs[j % n_chains]
                pv = p.bitcast(F32) if cfg["p_dtype"] == "f32r" else p
                with nc.allow_low_precision(reason="16-bit rowsum chains"):
                    if j < n_chains:
                        nc.vector.tensor_copy(acc, pv)
                    else:
                        nc.vector.tensor_add(acc, acc, pv)
                # rowsums: once chain i saw its last chunk, transpose-reduce
                # acc_i^T @ ones into its own psr columns (independent
                # accumulation groups; host sums the chains). n lands on
                # partitions, transposed for free.
                ci = j - (NCH - n_chains)
                if ci == 0:
                    psr = misc_tile("psr")
                if ci >= 0:
                    for nb in range(NQ // 128):
                        nc.tensor.matmul(
                            psr[:, ci * 8 + nb:ci * 8 + nb + 1],
                            lhsT=accs[ci][:, nb * 128:(nb + 1) * 128],
                            rhs=ones_p, start=True, stop=True)

            # ---- O^T = Wv @ Z (unnormalized; host divides by rowsums),
            # quarter-split so drain/matmul/copy/DMA pipeline across engines.
            # Rowsums ride along in cols [NQ, NQ+8) of the same output. ----
            z_sb = proj.tile([C, NQ], BF16, name="z_sb")
            pso = misc_tile("pso")
            nrs = (NQ // 128) * n_chains
            o_sb = owork.tile([128, NQ + nrs], F32, tag="o_sb", bufs=2,
                              name="o_sb")
            nc.vector.tensor_copy(o_sb[:, NQ:NQ + nrs], psr[:, 0:nrs])
            for q in range(4):
                qs = slice(q * 256, (q + 1) * 256)
                with nc.allow_low_precision(reason="bf16 z"):
                    if cfg["z_on_act"]:
                        nc.scalar.copy(z_sb[:, qs], psz[:, qs])
                    else:
                        nc.vector.tensor_copy(z_sb[:, qs], psz[:, qs])
                nc.tensor.matmul(pso[:, qs], lhsT=wvT_sb, rhs=z_sb[:, qs],
                                 start=True, stop=True)
                if q % 2 == 0:
                    nc.vector.tensor_copy(o_sb[:, qs], pso[:, qs])
                else:
                    nc.scalar.copy(o_sb[:, qs], pso[:, qs])
                if q < 3:
                    nc.sync.dma_start(out2[:, qs], o_sb[:, qs])
                else:
                    nc.sync.dma_start(out2[:, 768:NQ + nrs],
                                      o_sb[:, 768:NQ + nrs])
        if loop_cm is not None:
            loop_cm.__exit__(None, None, None)
    nc.compile()
    return nc


def _prepare_in_maps(query, key_value, Wq, bq, Wk, bk, Wv, bv, p_dtype=None):
    bf = ml_dtypes.bfloat16
    p_np = _P_NP[p_dtype or CFG["p_dtype"]]
    q = np.asarray(query, np.float32)
    kv = np.asarray(key_value, np.float32)
    Wq64 = np.asarray(Wq, np.float64)
    Wk64 = np.asarray(Wk, np.float64)
    aT = np.ascontiguousarray((Wq64.T @ Wk64).astype(bf))
    bqk = np.ascontiguousarray(
        (Wk64.T @ np.asarray(bq, np.float64)).astype(np.float32).reshape(C, 1)
    )
    wvT = np.ascontiguousarray(np.asarray(Wv, np.float32).T.astype(bf))
    kv_b = {}
    for b in range(B):
        kvx = kv[:, b].reshape(T, C, NQ).transpose(1, 0, 2).reshape(C, M)
        kvxT = kvx.T.reshape(NCH, 128, C).transpose(1, 0, 2).reshape(128, M)
        kv_b[b] = (
            np.ascontiguousarray(kvx.astype(bf)),
            np.ascontiguousarray(kvxT.astype(p_np)),
        )
    in_maps = []
    for core in range(N_CORES):
        b, t = divmod(core, T)
        qx = np.ascontiguousarray(q[t, b].reshape(C, NQ).astype(bf))
        in_maps.append({
            "qx": qx, "aT": aT, "bqk": bqk, "wvT": wvT,
            "kvx": kv_b[b][0], "kvxT": kv_b[b][1],
        })
    return in_maps


def _assemble(results, bv):
    full = np.empty((B, T * NQ, C), np.float32)
    for core in range(N_CORES):
        b, t = divmod(core, T)
        o2 = results[core]["out2"]
        oT = o2[:, :NQ]                              # [C, NQ] unnormalized
        rsv = o2[:, NQ:]                             # [p, chain*8+nb] partial sums
        nch = rsv.shape[1] // (NQ // 128)
        r = sum(rsv[:, ci * 8:(ci + 1) * 8] for ci in range(nch))
        r = r.T.reshape(NQ)                          # [p, nb] = rowsum(nb*128+p)
        full[b, t * NQ:(t + 1) * NQ] = (oT / r[None, :]).T
    full += np.asarray(bv, np.float32)[None, None, :]
    return full


def kernel(query, key_value, Wq, bq, Wk, bk, Wv, bv, **run_kwargs):
    global _NC
    if _NC is None:
        _NC = build_nc()
    in_maps = _prepare_in_maps(query, key_value, Wq, bq, Wk, bk, Wv, bv)
    res = run_bass_kernel_spmd(_NC, in_maps, list(range(N_CORES)), **run_kwargs)
    out = _assemble(res.results, bv)
    if run_kwargs:
        return out, res
    return out


# revision 17
# speedup vs baseline: 1.0114x; 1.0114x over previous
"""Fused cross-attention kernel for Trainium2 (Bass/Tile), 8-core SPMD.

Problem: query/key_value [T=4, B=2, C=128, H=32, W=32] -> tokens [B, N=4096, C],
QKV projections (128x128), full softmax attention over N tokens per batch.

Sharding: core = b*4 + t handles batch b, query tokens [t*1024, (t+1)*1024)
against all 4096 K/V tokens of batch b.

Algebraic restructure (vs. materializing Q/K/V):
  scores:  S^T[m,n] = x_kv[m] . qk[n]   with  qk = (Wk^T Wq) x_q + Wk^T bq
           (A = Wk^T Wq precomputed on host; bk shifts all scores of a row
           equally and drops out of softmax exactly)
  output:  O^T = Wv Z / rowsum,  Z[c,n] = sum_m x_kv[m,c] P[m,n]
           (V-projection pulled out of the attention sum by linearity)
So the device only runs: one 128x128 projection (qk), the two big
attention matmuls (S and Z), one final 128x128 matmul (Wv Z), exp, and
16-bit rowsum accumulation. No K/V projection matmuls, no per-chunk
PSUM->SBUF projection copies.

Per m-chunk (128 kv tokens, 32 chunks):
  pss [m=128, n=1024] = kvx_chunk^T @ qk       (PE, bf16, 2x512-col matmuls)
  p   = exp(SCALE * pss)                       (ACT, PSUM->SBUF, 16-bit out)
  psz [c=128, n=1024] += kvxT_chunk^T @ p      (PE, accumulated over chunks)
  acc_i += p                                   (DVE 2-byte 2x-mode adds)
Rowsums land pre-transposed via tiny PE matmuls acc_i^T @ ones accumulated
in PSUM [n-part, nb]; normalization and the final [C,NQ]->[NQ,C] transpose
happen on host (host already assembles shards and adds bv).

ldweights-only filler instructions (no PSUM write, no semaphores) can be
interleaved to keep the PE busy streak alive for the HAM clock ramp.

Inputs prepacked bf16 on host; P is 16-bit (exp <= e^7.7 ~ 2200 fits both
f16/bf16; validated ~3.6e-3 rel err end-to-end vs the 2e-2 gate).
"""

import math
from contextlib import ExitStack

import numpy as np
import ml_dtypes

import concourse.bass as bass
import concourse.mybir as mybir
import concourse.tile as tile
from concourse import bacc
from concourse.bass_utils import run_bass_kernel_spmd

F32 = mybir.dt.float32
F32R = mybir.dt.float32r
BF16 = mybir.dt.bfloat16
F16 = mybir.dt.float16
AF = mybir.ActivationFunctionType

C = 128        # model dim
NQ = 1024      # query tokens per core
M = 4096       # kv tokens per batch
NCH = M // 128 # m chunks
T = 4
B = 2
SCALE = 1.0 / math.sqrt(float(C))
N_CORES = 8

CFG = dict(
    p_dtype="f16",  # "f16" | "bf16" | "f32r": exp output / kvxT / rowsum dtype
    p_bufs=8,       # exp output SBUF buffers
    ps_s_bufs=3,    # score PSUM buffers ([128,1024] = 2 banks each)
    pe_warm=24,     # ldweights warm-ups during the DMA window
    fillers=2,      # ldweights fillers per chunk (hold the PE HAM streak)
    head_fill=8,    # ldweights fillers between qk proj and chunk 0
    z_on_act=True,  # drain Z psum->sbuf on ACT (idle after last exp)
    fillers_dep=True,  # fillers read p (un-hoistable, interleave per chunk)
    unroll=4,       # kernel bodies per For_i iteration (amortizes barrier)
)

_P_DT = {"f16": F16, "bf16": BF16, "f32r": F32R}
_P_NP = {"f16": np.float16, "bf16": ml_dtypes.bfloat16, "f32r": np.float32}
_N_CHAINS = {"f16": 2, "bf16": 4, "f32r": 2}

_NC = None


def build_nc(reps=1, loop_reps=0, **overrides):
    cfg = dict(CFG)
    cfg.update(overrides)
    p_dt = _P_DT[cfg["p_dtype"]]
    acc_dt = F32 if cfg["p_dtype"] == "f32r" else p_dt
    n_chains = _N_CHAINS[cfg["p_dtype"]]

    nc = bacc.Bacc()
    qx = nc.dram_tensor("qx", [C, NQ], BF16, kind="ExternalInput")
    aT = nc.dram_tensor("aT", [C, C], BF16, kind="ExternalInput")
    bqk = nc.dram_tensor("bqk", [C, 1], F32, kind="ExternalInput")
    wvT = nc.dram_tensor("wvT", [C, C], BF16, kind="ExternalInput")
    kvx = nc.dram_tensor("kvx", [C, M], BF16, kind="ExternalInput")
    kvxT = nc.dram_tensor("kvxT", [C, M], p_dt, kind="ExternalInput")
    out2 = nc.dram_tensor("out2", [C, NQ + (NQ // C) * n_chains], F32,
                          kind="ExternalOutput")

    unroll = cfg["unroll"]
    if loop_reps and loop_reps % unroll == 0 and loop_reps >= unroll:
        loop_iters, reps = loop_reps // unroll, reps * unroll
    elif loop_reps:
        loop_iters = loop_reps
    else:
        loop_iters = 0

    with tile.TileContext(nc) as tc, ExitStack() as ctx:
        const = ctx.enter_context(tc.tile_pool(name="const", bufs=1))
        proj = ctx.enter_context(tc.tile_pool(name="proj", bufs=1))
        pwork = ctx.enter_context(tc.tile_pool(name="pwork", bufs=cfg["p_bufs"]))
        owork = ctx.enter_context(tc.tile_pool(name="owork", bufs=2))
        psum = ctx.enter_context(tc.tile_pool(name="psum", bufs=2, space="PSUM"))

        def misc_tile(name):
            # borrow a rotating score-PSUM buffer for small/late matmuls
            return psum.tile([128, NQ], F32, tag="ps_s",
                             bufs=cfg["ps_s_bufs"], name=name)

        # Constants (gpsimd, no DMA deps). Warm the exp table first so the
        # 1.5us table load overlaps the input DMAs / NEFF preamble.
        ones_f32 = const.tile([128, 1], F32)
        nc.gpsimd.memset(ones_f32, 1.0)
        warm = const.tile([128, 1], F32)
        nc.scalar.activation(warm, ones_f32, AF.Exp)
        ones_p = const.tile([128, 1], acc_dt)
        nc.gpsimd.memset(ones_p, 1.0)
        warm_w = const.tile([128, 128], BF16)
        nc.gpsimd.memset(warm_w, 1.0)

        # PE warm-up: ldweights-only ops (no PSUM, no cross-engine deps)
        # to build a continuous-busy streak for the HAM clock ramp.
        for _w in range(cfg["pe_warm"]):
            nc.tensor.ldweights(warm_w)

        # Input DMAs, spread across the SP and ACT HWDGE rings.
        qx_sb = const.tile([C, NQ], BF16)
        nc.sync.dma_start(qx_sb, qx[:])
        aT_sb = const.tile([C, C], BF16)
        nc.sync.dma_start(aT_sb, aT[:])
        bqk_sb = const.tile([C, 1], F32)
        nc.sync.dma_start(bqk_sb, bqk[:])
        wvT_sb = const.tile([C, C], BF16)
        nc.sync.dma_start(wvT_sb, wvT[:])
        kvx_sb = const.tile([C, M], BF16)
        nc.sync.dma_start(kvx_sb[:, 0:1024], kvx[:, 0:1024])
        nc.scalar.dma_start(kvx_sb[:, 1024:2560], kvx[:, 1024:2560])
        nc.scalar.dma_start(kvx_sb[:, 2560:4096], kvx[:, 2560:4096])
        kvxT_sb = const.tile([C, M], p_dt)
        nc.sync.dma_start(kvxT_sb[:, 0:2048], kvxT[:, 0:2048])
        nc.scalar.dma_start(kvxT_sb[:, 2048:4096], kvxT[:, 2048:4096])

        loop_cm = tc.For_i(0, loop_iters, 1) if loop_iters else None
        if loop_cm is not None:
            loop_cm.__enter__()
        for _rep in range(reps):
            # ---- qk projection: qk = A @ qx + bqk ----
            # (matmul outputs may not cross a PSUM bank: 512 f32 cols max)
            psq = misc_tile("psq")
            for h in range(2):
                nc.tensor.matmul(psq[:, h * 512:(h + 1) * 512], lhsT=aT_sb,
                                 rhs=qx_sb[:, h * 512:(h + 1) * 512],
                                 start=True, stop=True)
            qk_sb = proj.tile([C, NQ], BF16, name="qk_sb")
            with nc.allow_low_precision(reason="bf16 qk tokens"):
                for h in range(2):
                    nc.vector.tensor_scalar_add(
                        qk_sb[:, h * 512:(h + 1) * 512],
                        psq[:, h * 512:(h + 1) * 512], bqk_sb)
            for _f in range(cfg["head_fill"]):
                nc.tensor.ldweights(warm_w)

            # ---- chunk loop ----
            psz = psum.tile([128, NQ], F32, tag="ps_z", bufs=1, name="psz")
            accs = [owork.tile([128, NQ], acc_dt, tag=f"acc{i}", bufs=1,
                               name=f"acc{i}") for i in range(n_chains)]
            for j in range(NCH):
                pss = psum.tile([128, NQ], F32, tag="ps_s",
                                bufs=cfg["ps_s_bufs"])
                for h in range(2):
                    nc.tensor.matmul(pss[:, h * 512:(h + 1) * 512],
                                     lhsT=kvx_sb[:, j * 128:(j + 1) * 128],
                                     rhs=qk_sb[:, h * 512:(h + 1) * 512],
                                     start=True, stop=True)
                if cfg["fillers"] and not cfg["fillers_dep"]:
                    for _f in range((cfg["fillers"] + 1) // 2):
                        nc.tensor.ldweights(warm_w)
                p = pwork.tile([128, NQ], p_dt, tag="p_sb", bufs=cfg["p_bufs"])
                nc.scalar.activation(p, pss, AF.Exp, scale=SCALE)
                for h in range(2):
                    nc.tensor.matmul(psz[:, h * 512:(h + 1) * 512],
                                     lhsT=kvxT_sb[:, j * 128:(j + 1) * 128],
                                     rhs=p[:, h * 512:(h + 1) * 512],
                                     start=(j == 0), stop=(j == NCH - 1))
                if cfg["fillers"]:
                    nf = (cfg["fillers"] if cfg["fillers_dep"]
                          else cfg["fillers"] // 2)
                    for _f in range(nf):
                        # reading p makes the filler depend on this chunk's
                        # exp, so the scheduler cannot hoist it into a blob
                        nc.tensor.ldweights(p[:, _f * 128:(_f + 1) * 128])
                acc = accs[j % n_chains]
                pv = p.bitcast(F32) if cfg["p_dtype"] == "f32r" else p
                with nc.allow_low_precision(reason="16-bit rowsum chains"):
                    if j < n_chains:
                        nc.vector.tensor_copy(acc, pv)
                    else:
                        nc.vector.tensor_add(acc, acc, pv)
                # rowsums: once chain i saw its last chunk, transpose-reduce
                # acc_i^T @ ones into its own psr columns (independent
                # accumulation groups; host sums the chains). n lands on
                # partitions, transposed for free.
                ci = j - (NCH - n_chains)
                if ci == 0:
                    psr = misc_tile("psr")
                if ci >= 0:
                    for nb in range(NQ // 128):
                        nc.tensor.matmul(
                            psr[:, ci * 8 + nb:ci * 8 + nb + 1],
                            lhsT=accs[ci][:, nb * 128:(nb + 1) * 128],
                            rhs=ones_p, start=True, stop=True)

            # ---- O^T = Wv @ Z (unnormalized; host divides by rowsums),
            # quarter-split so drain/matmul/copy/DMA pipeline across engines.
            # Rowsums ride along in cols [NQ, NQ+8) of the same output. ----
            z_sb = proj.tile([C, NQ], BF16, name="z_sb")
            pso = misc_tile("pso")
            nrs = (NQ // 128) * n_chains
            o_sb = owork.tile([128, NQ + nrs], F32, tag="o_sb", bufs=2,
                              name="o_sb")
            nc.vector.tensor_copy(o_sb[:, NQ:NQ + nrs], psr[:, 0:nrs])
            for q in range(4):
                qs = slice(q * 256, (q + 1) * 256)
                with nc.allow_low_precision(reason="bf16 z"):
                    if cfg["z_on_act"]:
                        nc.scalar.copy(z_sb[:, qs], psz[:, qs])
                    else:
                        nc.vector.tensor_copy(z_sb[:, qs], psz[:, qs])
                nc.tensor.matmul(pso[:, qs], lhsT=wvT_sb, rhs=z_sb[:, qs],
                                 start=True, stop=True)
                if q % 2 == 0:
                    nc.vector.tensor_copy(o_sb[:, qs], pso[:, qs])
                else:
                    nc.scalar.copy(o_sb[:, qs], pso[:, qs])
                if q < 3:
                    nc.sync.dma_start(out2[:, qs], o_sb[:, qs])
                else:
                    nc.sync.dma_start(out2[:, 768:NQ + nrs],
                                      o_sb[:, 768:NQ + nrs])
        if loop_cm is not None:
            loop_cm.__exit__(None, None, None)
    nc.compile()
    return nc


def _prepare_in_maps(query, key_value, Wq, bq, Wk, bk, Wv, bv, p_dtype=None):
    bf = ml_dtypes.bfloat16
    p_np = _P_NP[p_dtype or CFG["p_dtype"]]
    q = np.asarray(query, np.float32)
    kv = np.asarray(key_value, np.float32)
    Wq64 = np.asarray(Wq, np.float64)
    Wk64 = np.asarray(Wk, np.float64)
    aT = np.ascontiguousarray((Wq64.T @ Wk64).astype(bf))
    bqk = np.ascontiguousarray(
        (Wk64.T @ np.asarray(bq, np.float64)).astype(np.float32).reshape(C, 1)
    )
    wvT = np.ascontiguousarray(np.asarray(Wv, np.float32).T.astype(bf))
    kv_b = {}
    for b in range(B):
        kvx = kv[:, b].reshape(T, C, NQ).transpose(1, 0, 2).reshape(C, M)
        kvxT = kvx.T.reshape(NCH, 128, C).transpose(1, 0, 2).reshape(128, M)
        kv_b[b] = (
            np.ascontiguousarray(kvx.astype(bf)),
            np.ascontiguousarray(kvxT.astype(p_np)),
        )
    in_maps = []
    for core in range(N_CORES):
        b, t = divmod(core, T)
        qx = np.ascontiguousarray(q[t, b].reshape(C, NQ).astype(bf))
        in_maps.append({
            "qx": qx, "aT": aT, "bqk": bqk, "wvT": wvT,
            "kvx": kv_b[b][0], "kvxT": kv_b[b][1],
        })
    return in_maps


def _assemble(results, bv):
    full = np.empty((B, T * NQ, C), np.float32)
    for core in range(N_CORES):
        b, t = divmod(core, T)
        o2 = results[core]["out2"]
        oT = o2[:, :NQ]                              # [C, NQ] unnormalized
        rsv = o2[:, NQ:]                             # [p, chain*8+nb] partial sums
        nch = rsv.shape[1] // (NQ // 128)
        r = sum(rsv[:, ci * 8:(ci + 1) * 8] for ci in range(nch))
        r = r.T.reshape(NQ)                          # [p, nb] = rowsum(nb*128+p)
        full[b, t * NQ:(t + 1) * NQ] = (oT / r[None, :]).T
    full += np.asarray(bv, np.float32)[None, None, :]
    return full


def kernel(query, key_value, Wq, bq, Wk, bk, Wv, bv, **run_kwargs):
    global _NC
    if _NC is None:
        _NC = build_nc()
    in_maps = _prepare_in_maps(query, key_value, Wq, bq, Wk, bk, Wv, bv)
    res = run_bass_kernel_spmd(_NC, in_maps, list(range(N_CORES)), **run_kwargs)
    out = _assemble(res.results, bv)
    if run_kwargs:
        return out, res
    return out


# revision 18
# speedup vs baseline: 1.1648x; 1.1517x over previous
"""Fused cross-attention kernel for Trainium2 (Bass/Tile), 8-core SPMD.

Problem: query/key_value [T=4, B=2, C=128, H=32, W=32] -> tokens [B, N=4096, C],
QKV projections (128x128), full softmax attention over N tokens per batch.

Sharding: core = b*4 + t handles batch b, query tokens [t*1024, (t+1)*1024)
against all 4096 K/V tokens of batch b.

Algebraic restructure (vs. materializing Q/K/V):
  scores:  S^T[m,n] = x_kv[m] . qk[n]   with  qk = (Wk^T Wq) x_q + Wk^T bq
           (A = Wk^T Wq precomputed on host; bk shifts all scores of a row
           equally and drops out of softmax exactly)
  output:  O^T = Wv Z / rowsum,  Z[c,n] = sum_m x_kv[m,c] P[m,n]
           (V-projection pulled out of the attention sum by linearity)
So the device only runs: one 128x128 projection (qk), the two big
attention matmuls (S and Z), one final 128x128 matmul (Wv Z), exp, and
16-bit rowsum accumulation. No K/V projection matmuls, no per-chunk
PSUM->SBUF projection copies.

Per m-chunk (128 kv tokens, 32 chunks):
  pss [m=128, n=1024] = kvx_chunk^T @ qk       (PE, bf16, 2x512-col matmuls)
  p   = exp(SCALE * pss)                       (ACT, PSUM->SBUF, 16-bit out)
  psz [c=128, n=1024] += kvxT_chunk^T @ p      (PE, accumulated over chunks)
  acc_i += p                                   (DVE 2-byte 2x-mode adds)
Rowsums land pre-transposed via tiny PE matmuls acc_i^T @ ones accumulated
in PSUM [n-part, nb]; normalization and the final [C,NQ]->[NQ,C] transpose
happen on host (host already assembles shards and adds bv).

ldweights-only filler instructions (no PSUM write, no semaphores) can be
interleaved to keep the PE busy streak alive for the HAM clock ramp.

Inputs prepacked bf16 on host; P is 16-bit (exp <= e^7.7 ~ 2200 fits both
f16/bf16; validated ~3.6e-3 rel err end-to-end vs the 2e-2 gate).
"""

import math
from contextlib import ExitStack

import numpy as np
import ml_dtypes

import concourse.bass as bass
import concourse.mybir as mybir
import concourse.tile as tile
from concourse import bacc
from concourse.bass_utils import run_bass_kernel_spmd

F32 = mybir.dt.float32
F32R = mybir.dt.float32r
BF16 = mybir.dt.bfloat16
F16 = mybir.dt.float16
AF = mybir.ActivationFunctionType

C = 128        # model dim
NQ = 1024      # query tokens per core
M = 4096       # kv tokens per batch
NCH = M // 128 # m chunks
T = 4
B = 2
SCALE = 1.0 / math.sqrt(float(C))
N_CORES = 8

CFG = dict(
    p_dtype="f16",  # "f16" | "bf16" | "f32r": exp output / kvxT / rowsum dtype
    p_bufs=8,       # exp output SBUF buffers
    ps_s_bufs=3,    # score PSUM buffers ([128,1024] = 2 banks each)
    pe_warm=24,     # ldweights warm-ups during the DMA window
    fillers=2,      # ldweights fillers per chunk (hold the PE HAM streak)
    head_fill=8,    # ldweights fillers between qk proj and chunk 0
    z_on_act=True,  # drain Z psum->sbuf on ACT (idle after last exp)
    fillers_dep=True,  # fillers read p (un-hoistable, interleave per chunk)
    unroll=4,       # kernel bodies per For_i iteration (amortizes barrier)
)

_P_DT = {"f16": F16, "bf16": BF16, "f32r": F32R}
_P_NP = {"f16": np.float16, "bf16": ml_dtypes.bfloat16, "f32r": np.float32}
_N_CHAINS = {"f16": 2, "bf16": 4, "f32r": 2}

_NC = None


def build_nc(reps=1, loop_reps=0, **overrides):
    cfg = dict(CFG)
    cfg.update(overrides)
    p_dt = _P_DT[cfg["p_dtype"]]
    acc_dt = F32 if cfg["p_dtype"] == "f32r" else p_dt
    n_chains = _N_CHAINS[cfg["p_dtype"]]

    nc = bacc.Bacc()
    qx = nc.dram_tensor("qx", [C, NQ], BF16, kind="ExternalInput")
    aT = nc.dram_tensor("aT", [C, C], BF16, kind="ExternalInput")
    bqk = nc.dram_tensor("bqk", [C, 1], F32, kind="ExternalInput")
    wvT = nc.dram_tensor("wvT", [C, C], BF16, kind="ExternalInput")
    kvx = nc.dram_tensor("kvx", [C, M], BF16, kind="ExternalInput")
    kvxT = nc.dram_tensor("kvxT", [C, M], p_dt, kind="ExternalInput")
    out2 = nc.dram_tensor("out2", [C, NQ + (NQ // C) * n_chains], F32,
                          kind="ExternalOutput")

    unroll = cfg["unroll"]
    if loop_reps and loop_reps % unroll == 0 and loop_reps >= unroll:
        loop_iters, reps = loop_reps // unroll, reps * unroll
    elif loop_reps:
        loop_iters = loop_reps
    else:
        loop_iters = 0

    with tile.TileContext(nc) as tc, ExitStack() as ctx:
        const = ctx.enter_context(tc.tile_pool(name="const", bufs=1))
        proj = ctx.enter_context(tc.tile_pool(name="proj", bufs=1))
        pwork = ctx.enter_context(tc.tile_pool(name="pwork", bufs=cfg["p_bufs"]))
        owork = ctx.enter_context(tc.tile_pool(name="owork", bufs=2))
        psum = ctx.enter_context(tc.tile_pool(name="psum", bufs=2, space="PSUM"))

        def misc_tile(name):
            # borrow a rotating score-PSUM buffer for small/late matmuls
            return psum.tile([128, NQ], F32, tag="ps_s",
                             bufs=cfg["ps_s_bufs"], name=name)

        # Constants (gpsimd, no DMA deps). Warm the exp table first so the
        # 1.5us table load overlaps the input DMAs / NEFF preamble.
        ones_f32 = const.tile([128, 1], F32)
        nc.gpsimd.memset(ones_f32, 1.0)
        warm = const.tile([128, 1], F32)
        nc.scalar.activation(warm, ones_f32, AF.Exp)
        ones_p = const.tile([128, 1], acc_dt)
        nc.gpsimd.memset(ones_p, 1.0)
        warm_w = const.tile([128, 128], BF16)
        nc.gpsimd.memset(warm_w, 1.0)

        # PE warm-up: ldweights-only ops (no PSUM, no cross-engine deps)
        # to build a continuous-busy streak for the HAM clock ramp.
        for _w in range(cfg["pe_warm"]):
            nc.tensor.ldweights(warm_w)

        # Input DMAs, spread across the SP and ACT HWDGE rings.
        qx_sb = const.tile([C, NQ], BF16)
        nc.sync.dma_start(qx_sb, qx[:])
        aT_sb = const.tile([C, C], BF16)
        nc.sync.dma_start(aT_sb, aT[:])
        bqk_sb = const.tile([C, 1], F32)
        nc.sync.dma_start(bqk_sb, bqk[:])
        wvT_sb = const.tile([C, C], BF16)
        nc.sync.dma_start(wvT_sb, wvT[:])
        kvx_sb = const.tile([C, M], BF16)
        nc.sync.dma_start(kvx_sb[:, 0:1024], kvx[:, 0:1024])
        nc.scalar.dma_start(kvx_sb[:, 1024:2560], kvx[:, 1024:2560])
        nc.scalar.dma_start(kvx_sb[:, 2560:4096], kvx[:, 2560:4096])
        kvxT_sb = const.tile([C, M], p_dt)
        nc.sync.dma_start(kvxT_sb[:, 0:2048], kvxT[:, 0:2048])
        nc.scalar.dma_start(kvxT_sb[:, 2048:4096], kvxT[:, 2048:4096])

        loop_cm = tc.For_i(0, loop_iters, 1) if loop_iters else None
        if loop_cm is not None:
            loop_cm.__enter__()
        for _rep in range(reps):
            # ---- qk projection: qk = A @ qx + bqk ----
            # (matmul outputs may not cross a PSUM bank: 512 f32 cols max)
            psq = misc_tile("psq")
            for h in range(2):
                nc.tensor.matmul(psq[:, h * 512:(h + 1) * 512], lhsT=aT_sb,
                                 rhs=qx_sb[:, h * 512:(h + 1) * 512],
                                 start=True, stop=True)
            qk_sb = proj.tile([C, NQ], BF16, name="qk_sb")
            with nc.allow_low_precision(reason="bf16 qk tokens"):
                for h in range(2):
                    nc.vector.tensor_scalar_add(
                        qk_sb[:, h * 512:(h + 1) * 512],
                        psq[:, h * 512:(h + 1) * 512], bqk_sb)
            for _f in range(cfg["head_fill"]):
                nc.tensor.ldweights(warm_w)

            # ---- chunk loop ----
            psz = psum.tile([128, NQ], F32, tag="ps_z", bufs=1, name="psz")
            accs = [owork.tile([128, NQ], acc_dt, tag=f"acc{i}", bufs=1,
                               name=f"acc{i}") for i in range(n_chains)]
            for j in range(NCH):
                pss = psum.tile([128, NQ], F32, tag="ps_s",
                                bufs=cfg["ps_s_bufs"])
                for h in range(2):
                    nc.tensor.matmul(pss[:, h * 512:(h + 1) * 512],
                                     lhsT=kvx_sb[:, j * 128:(j + 1) * 128],
                                     rhs=qk_sb[:, h * 512:(h + 1) * 512],
                                     start=True, stop=True)
                if cfg["fillers"] and not cfg["fillers_dep"]:
                    for _f in range((cfg["fillers"] + 1) // 2):
                        nc.tensor.ldweights(warm_w)
                p = pwork.tile([128, NQ], p_dt, tag="p_sb", bufs=cfg["p_bufs"])
                nc.scalar.activation(p, pss, AF.Exp, scale=SCALE)
                for h in range(2):
                    nc.tensor.matmul(psz[:, h * 512:(h + 1) * 512],
                                     lhsT=kvxT_sb[:, j * 128:(j + 1) * 128],
                                     rhs=p[:, h * 512:(h + 1) * 512],
                                     start=(j == 0), stop=(j == NCH - 1))
                if cfg["fillers"]:
                    nf = (cfg["fillers"] if cfg["fillers_dep"]
                          else cfg["fillers"] // 2)
                    for _f in range(nf):
                        # reading p makes the filler depend on this chunk's
                        # exp, so the scheduler cannot hoist it into a blob
                        nc.tensor.ldweights(p[:, _f * 128:(_f + 1) * 128])
                acc = accs[j % n_chains]
                pv = p.bitcast(F32) if cfg["p_dtype"] == "f32r" else p
                with nc.allow_low_precision(reason="16-bit rowsum chains"):
                    if j < n_chains:
                        nc.vector.tensor_copy(acc, pv)
                    else:
                        nc.vector.tensor_add(acc, acc, pv)
                # rowsums: once chain i saw its last chunk, transpose-reduce
                # acc_i^T @ ones into its own psr columns (independent
                # accumulation groups; host sums the chains). n lands on
                # partitions, transposed for free.
                ci = j - (NCH - n_chains)
                if ci == 0:
                    psr = misc_tile("psr")
                if ci >= 0:
                    for nb in range(NQ // 128):
                        nc.tensor.matmul(
                            psr[:, ci * 8 + nb:ci * 8 + nb + 1],
                            lhsT=accs[ci][:, nb * 128:(nb + 1) * 128],
                            rhs=ones_p, start=True, stop=True)

            # ---- O^T = Wv @ Z (unnormalized; host divides by rowsums),
            # quarter-split so drain/matmul/copy/DMA pipeline across engines.
            # Rowsums ride along in cols [NQ, NQ+8) of the same output. ----
            z_sb = proj.tile([C, NQ], BF16, name="z_sb")
            pso = misc_tile("pso")
            nrs = (NQ // 128) * n_chains
            o_sb = owork.tile([128, NQ + nrs], F32, tag="o_sb", bufs=2,
                              name="o_sb")
            nc.vector.tensor_copy(o_sb[:, NQ:NQ + nrs], psr[:, 0:nrs])
            for q in range(4):
                qs = slice(q * 256, (q + 1) * 256)
                with nc.allow_low_precision(reason="bf16 z"):
                    if cfg["z_on_act"]:
                        nc.scalar.copy(z_sb[:, qs], psz[:, qs])
                    else:
                        nc.vector.tensor_copy(z_sb[:, qs], psz[:, qs])
                nc.tensor.matmul(pso[:, qs], lhsT=wvT_sb, rhs=z_sb[:, qs],
                                 start=True, stop=True)
                nc.vector.tensor_copy(o_sb[:, qs], pso[:, qs])
                if q < 3:
                    nc.sync.dma_start(out2[:, qs], o_sb[:, qs])
                else:
                    nc.sync.dma_start(out2[:, 768:NQ + nrs],
                                      o_sb[:, 768:NQ + nrs])
        if loop_cm is not None:
            loop_cm.__exit__(None, None, None)
    nc.compile()
    return nc


def _prepare_in_maps(query, key_value, Wq, bq, Wk, bk, Wv, bv, p_dtype=None):
    bf = ml_dtypes.bfloat16
    p_np = _P_NP[p_dtype or CFG["p_dtype"]]
    q = np.asarray(query, np.float32)
    kv = np.asarray(key_value, np.float32)
    Wq64 = np.asarray(Wq, np.float64)
    Wk64 = np.asarray(Wk, np.float64)
    aT = np.ascontiguousarray((Wq64.T @ Wk64).astype(bf))
    bqk = np.ascontiguousarray(
        (Wk64.T @ np.asarray(bq, np.float64)).astype(np.float32).reshape(C, 1)
    )
    wvT = np.ascontiguousarray(np.asarray(Wv, np.float32).T.astype(bf))
    kv_b = {}
    for b in range(B):
        kvx = kv[:, b].reshape(T, C, NQ).transpose(1, 0, 2).reshape(C, M)
        kvxT = kvx.T.reshape(NCH, 128, C).transpose(1, 0, 2).reshape(128, M)
        kv_b[b] = (
            np.ascontiguousarray(kvx.astype(bf)),
            np.ascontiguousarray(kvxT.astype(p_np)),
        )
    in_maps = []
    for core in range(N_CORES):
        b, t = divmod(core, T)
        qx = np.ascontiguousarray(q[t, b].reshape(C, NQ).astype(bf))
        in_maps.append({
            "qx": qx, "aT": aT, "bqk": bqk, "wvT": wvT,
            "kvx": kv_b[b][0], "kvxT": kv_b[b][1],
        })
    return in_maps


def _assemble(results, bv):
    full = np.empty((B, T * NQ, C), np.float32)
    for core in range(N_CORES):
        b, t = divmod(core, T)
        o2 = results[core]["out2"]
        oT = o2[:, :NQ]                              # [C, NQ] unnormalized
        rsv = o2[:, NQ:]                             # [p, chain*8+nb] partial sums
        nch = rsv.shape[1] // (NQ // 128)
        r = sum(rsv[:, ci * 8:(ci + 1) * 8] for ci in range(nch))
        r = r.T.reshape(NQ)                          # [p, nb] = rowsum(nb*128+p)
        full[b, t * NQ:(t + 1) * NQ] = (oT / r[None, :]).T
    full += np.asarray(bv, np.float32)[None, None, :]
    return full


def kernel(query, key_value, Wq, bq, Wk, bk, Wv, bv, **run_kwargs):
    global _NC
    if _NC is None:
        _NC = build_nc()
    in_maps = _prepare_in_maps(query, key_value, Wq, bq, Wk, bk, Wv, bv)
    res = run_bass_kernel_spmd(_NC, in_maps, list(range(N_CORES)), **run_kwargs)
    out = _assemble(res.results, bv)
    if run_kwargs:
        return out, res
    return out


# revision 19
# speedup vs baseline: 1.2489x; 1.0722x over previous
"""Fused cross-attention kernel for Trainium2 (Bass/Tile), 8-core SPMD.

Problem: query/key_value [T=4, B=2, C=128, H=32, W=32] -> tokens [B, N=4096, C],
QKV projections (128x128), full softmax attention over N tokens per batch.

Sharding: core = b*4 + t handles batch b, query tokens [t*1024, (t+1)*1024)
against all 4096 K/V tokens of batch b.

Algebraic restructure (vs. materializing Q/K/V):
  scores:  S^T[m,n] = x_kv[m] . qk[n]   with  qk = (Wk^T Wq) x_q + Wk^T bq
           (A = Wk^T Wq precomputed on host; bk shifts all scores of a row
           equally and drops out of softmax exactly)
  output:  O^T = Wv Z / rowsum,  Z[c,n] = sum_m x_kv[m,c] P[m,n]
           (V-projection pulled out of the attention sum by linearity)
So the device only runs: one 128x128 projection (qk), the two big
attention matmuls (S and Z), one final 128x128 matmul (Wv Z), exp, and
16-bit rowsum accumulation. No K/V projection matmuls, no per-chunk
PSUM->SBUF projection copies.

Per m-chunk (128 kv tokens, 32 chunks):
  pss [m=128, n=1024] = kvx_chunk^T @ qk       (PE, bf16, 2x512-col matmuls)
  p   = exp(SCALE * pss)                       (ACT, PSUM->SBUF, 16-bit out)
  psz [c=128, n=1024] += kvxT_chunk^T @ p      (PE, accumulated over chunks)
  acc_i += p                                   (DVE 2-byte 2x-mode adds)
Rowsums land pre-transposed via tiny PE matmuls acc_i^T @ ones accumulated
in PSUM [n-part, nb]; normalization and the final [C,NQ]->[NQ,C] transpose
happen on host (host already assembles shards and adds bv).

ldweights-only filler instructions (no PSUM write, no semaphores) can be
interleaved to keep the PE busy streak alive for the HAM clock ramp.

Inputs prepacked bf16 on host; P is 16-bit (exp <= e^7.7 ~ 2200 fits both
f16/bf16; validated ~3.6e-3 rel err end-to-end vs the 2e-2 gate).
"""

import math
from contextlib import ExitStack

import numpy as np
import ml_dtypes

import concourse.bass as bass
import concourse.mybir as mybir
import concourse.tile as tile
from concourse import bacc
from concourse.bass_utils import run_bass_kernel_spmd

F32 = mybir.dt.float32
F32R = mybir.dt.float32r
BF16 = mybir.dt.bfloat16
F16 = mybir.dt.float16
AF = mybir.ActivationFunctionType

C = 128        # model dim
NQ = 1024      # query tokens per core
M = 4096       # kv tokens per batch
NCH = M // 128 # m chunks
T = 4
B = 2
SCALE = 1.0 / math.sqrt(float(C))
N_CORES = 8

CFG = dict(
    p_dtype="f16",  # "f16" | "bf16" | "f32r": exp output / kvxT / rowsum dtype
    p_bufs=8,       # exp output SBUF buffers
    ps_s_bufs=3,    # score PSUM buffers ([128,1024] = 2 banks each)
    pe_warm=24,     # ldweights warm-ups during the DMA window
    fillers=2,      # ldweights fillers per chunk (hold the PE HAM streak)
    head_fill=8,    # ldweights fillers between qk proj and chunk 0
    z_on_act=True,  # drain Z psum->sbuf on ACT (idle after last exp)
    fillers_dep=True,  # fillers read p (un-hoistable, interleave per chunk)
    unroll=4,       # kernel bodies per For_i iteration (amortizes barrier)
)

_P_DT = {"f16": F16, "bf16": BF16, "f32r": F32R}
_P_NP = {"f16": np.float16, "bf16": ml_dtypes.bfloat16, "f32r": np.float32}
_N_CHAINS = {"f16": 2, "bf16": 4, "f32r": 2}

_NC = None


def build_nc(reps=1, loop_reps=0, **overrides):
    cfg = dict(CFG)
    cfg.update(overrides)
    p_dt = _P_DT[cfg["p_dtype"]]
    acc_dt = F32 if cfg["p_dtype"] == "f32r" else p_dt
    n_chains = _N_CHAINS[cfg["p_dtype"]]

    nc = bacc.Bacc()
    qx = nc.dram_tensor("qx", [C, NQ], BF16, kind="ExternalInput")
    aT = nc.dram_tensor("aT", [C, C], BF16, kind="ExternalInput")
    bqk = nc.dram_tensor("bqk", [C, 1], F32, kind="ExternalInput")
    wvT = nc.dram_tensor("wvT", [C, C], BF16, kind="ExternalInput")
    kvx = nc.dram_tensor("kvx", [C, M], BF16, kind="ExternalInput")
    kvxT = nc.dram_tensor("kvxT", [C, M], p_dt, kind="ExternalInput")
    out2 = nc.dram_tensor("out2", [C, NQ + (NQ // C) * n_chains], F32,
                          kind="ExternalOutput")

    unroll = cfg["unroll"]
    if loop_reps and loop_reps % unroll == 0 and loop_reps >= unroll:
        loop_iters, reps = loop_reps // unroll, reps * unroll
    elif loop_reps:
        loop_iters = loop_reps
    else:
        loop_iters = 0

    with tile.TileContext(nc) as tc, ExitStack() as ctx:
        const = ctx.enter_context(tc.tile_pool(name="const", bufs=1))
        proj = ctx.enter_context(tc.tile_pool(name="proj", bufs=1))
        pwork = ctx.enter_context(tc.tile_pool(name="pwork", bufs=cfg["p_bufs"]))
        owork = ctx.enter_context(tc.tile_pool(name="owork", bufs=2))
        psum = ctx.enter_context(tc.tile_pool(name="psum", bufs=2, space="PSUM"))

        def misc_tile(name):
            # borrow a rotating score-PSUM buffer for small/late matmuls
            return psum.tile([128, NQ], F32, tag="ps_s",
                             bufs=cfg["ps_s_bufs"], name=name)

        # Constants (gpsimd, no DMA deps). Warm the exp table first so the
        # 1.5us table load overlaps the input DMAs / NEFF preamble.
        ones_f32 = const.tile([128, 1], F32)
        nc.gpsimd.memset(ones_f32, 1.0)
        warm = const.tile([128, 1], F32)
        nc.scalar.activation(warm, ones_f32, AF.Exp)
        ones_p = const.tile([128, 1], acc_dt)
        nc.gpsimd.memset(ones_p, 1.0)
        warm_w = const.tile([128, 128], BF16)
        nc.gpsimd.memset(warm_w, 1.0)

        # PE warm-up: ldweights-only ops (no PSUM, no cross-engine deps)
        # to build a continuous-busy streak for the HAM clock ramp.
        for _w in range(cfg["pe_warm"]):
            nc.tensor.ldweights(warm_w)

        # Input DMAs, spread across the SP and ACT HWDGE rings.
        qx_sb = const.tile([C, NQ], BF16)
        nc.sync.dma_start(qx_sb, qx[:])
        aT_sb = const.tile([C, C], BF16)
        nc.sync.dma_start(aT_sb, aT[:])
        bqk_sb = const.tile([C, 1], F32)
        nc.sync.dma_start(bqk_sb, bqk[:])
        wvT_sb = const.tile([C, C], BF16)
        nc.sync.dma_start(wvT_sb, wvT[:])
        kvx_sb = const.tile([C, M], BF16)
        nc.sync.dma_start(kvx_sb[:, 0:1024], kvx[:, 0:1024])
        nc.scalar.dma_start(kvx_sb[:, 1024:2560], kvx[:, 1024:2560])
        nc.scalar.dma_start(kvx_sb[:, 2560:4096], kvx[:, 2560:4096])
        kvxT_sb = const.tile([C, M], p_dt)
        nc.sync.dma_start(kvxT_sb[:, 0:2048], kvxT[:, 0:2048])
        nc.scalar.dma_start(kvxT_sb[:, 2048:4096], kvxT[:, 2048:4096])

        loop_cm = tc.For_i(0, loop_iters, 1) if loop_iters else None
        if loop_cm is not None:
            loop_cm.__enter__()
        for _rep in range(reps):
            # ---- qk projection: qk = A @ qx + bqk ----
            # (matmul outputs may not cross a PSUM bank: 512 f32 cols max)
            psq = misc_tile("psq")
            for h in range(2):
                nc.tensor.matmul(psq[:, h * 512:(h + 1) * 512], lhsT=aT_sb,
                                 rhs=qx_sb[:, h * 512:(h + 1) * 512],
                                 start=True, stop=True)
            qk_sb = proj.tile([C, NQ], BF16, name="qk_sb")
            with nc.allow_low_precision(reason="bf16 qk tokens"):
                for h in range(2):
                    nc.vector.tensor_scalar_add(
                        qk_sb[:, h * 512:(h + 1) * 512],
                        psq[:, h * 512:(h + 1) * 512], bqk_sb)
            for _f in range(cfg["head_fill"]):
                nc.tensor.ldweights(warm_w)

            # ---- chunk loop ----
            psz = psum.tile([128, NQ], F32, tag="ps_z", bufs=1, name="psz")
            accs = [owork.tile([128, NQ], acc_dt, tag=f"acc{i}", bufs=1,
                               name=f"acc{i}") for i in range(n_chains)]
            for j in range(NCH):
                pss = psum.tile([128, NQ], F32, tag="ps_s",
                                bufs=cfg["ps_s_bufs"])
                for h in range(2):
                    nc.tensor.matmul(pss[:, h * 512:(h + 1) * 512],
                                     lhsT=kvx_sb[:, j * 128:(j + 1) * 128],
                                     rhs=qk_sb[:, h * 512:(h + 1) * 512],
                                     start=True, stop=True)
                if cfg["fillers"] and not cfg["fillers_dep"]:
                    for _f in range((cfg["fillers"] + 1) // 2):
                        nc.tensor.ldweights(warm_w)
                p = pwork.tile([128, NQ], p_dt, tag="p_sb", bufs=cfg["p_bufs"])
                nc.scalar.activation(p, pss, AF.Exp, scale=SCALE)
                for h in range(2):
                    nc.tensor.matmul(psz[:, h * 512:(h + 1) * 512],
                                     lhsT=kvxT_sb[:, j * 128:(j + 1) * 128],
                                     rhs=p[:, h * 512:(h + 1) * 512],
                                     start=(j == 0), stop=(j == NCH - 1))
                if cfg["fillers"]:
                    nf = (cfg["fillers"] if cfg["fillers_dep"]
                          else cfg["fillers"] // 2)
                    for _f in range(nf):
                        # reading p makes the filler depend on this chunk's
                        # exp, so the scheduler cannot hoist it into a blob
                        nc.tensor.ldweights(p[:, _f * 128:(_f + 1) * 128])
                acc = accs[j % n_chains]
                pv = p.bitcast(F32) if cfg["p_dtype"] == "f32r" else p
                with nc.allow_low_precision(reason="16-bit rowsum chains"):
                    if j < n_chains:
                        nc.vector.tensor_copy(acc, pv)
                    else:
                        nc.vector.tensor_add(acc, acc, pv)

            # ---- rowsums: transpose-reduce acc_i^T @ ones into per-chain
            # psr columns (independent groups; host sums the chains). n lands
            # on partitions for free. Acquired after the loop so the score-
            # PSUM rotation is not starved at chunks 30-31; the matmuls are
            # PE-side and run parallel to the ACT z-drain.
            psr = misc_tile("psr")
            for ci in range(n_chains):
                for nb in range(NQ // 128):
                    nc.tensor.matmul(
                        psr[:, ci * 8 + nb:ci * 8 + nb + 1],
                        lhsT=accs[ci][:, nb * 128:(nb + 1) * 128],
                        rhs=ones_p, start=True, stop=True)

            # ---- O^T = Wv @ Z (unnormalized; host divides by rowsums),
            # quarter-split so drain/matmul/copy/DMA pipeline across engines.
            # Rowsums ride along in cols [NQ, NQ+8) of the same output. ----
            z_sb = proj.tile([C, NQ], BF16, name="z_sb")
            pso = misc_tile("pso")
            nrs = (NQ // 128) * n_chains
            o_sb = owork.tile([128, NQ + nrs], F32, tag="o_sb", bufs=2,
                              name="o_sb")
            nc.vector.tensor_copy(o_sb[:, NQ:NQ + nrs], psr[:, 0:nrs])
            for q in range(4):
                qs = slice(q * 256, (q + 1) * 256)
                with nc.allow_low_precision(reason="bf16 z"):
                    if cfg["z_on_act"]:
                        nc.scalar.copy(z_sb[:, qs], psz[:, qs])
                    else:
                        nc.vector.tensor_copy(z_sb[:, qs], psz[:, qs])
                nc.tensor.matmul(pso[:, qs], lhsT=wvT_sb, rhs=z_sb[:, qs],
                                 start=True, stop=True)
                nc.vector.tensor_copy(o_sb[:, qs], pso[:, qs])
                if q < 3:
                    nc.sync.dma_start(out2[:, qs], o_sb[:, qs])
                else:
                    nc.sync.dma_start(out2[:, 768:NQ + nrs],
                                      o_sb[:, 768:NQ + nrs])
        if loop_cm is not None:
            loop_cm.__exit__(None, None, None)
    nc.compile()
    return nc


def _prepare_in_maps(query, key_value, Wq, bq, Wk, bk, Wv, bv, p_dtype=None):
    bf = ml_dtypes.bfloat16
    p_np = _P_NP[p_dtype or CFG["p_dtype"]]
    q = np.asarray(query, np.float32)
    kv = np.asarray(key_value, np.float32)
    Wq64 = np.asarray(Wq, np.float64)
    Wk64 = np.asarray(Wk, np.float64)
    aT = np.ascontiguousarray((Wq64.T @ Wk64).astype(bf))
    bqk = np.ascontiguousarray(
        (Wk64.T @ np.asarray(bq, np.float64)).astype(np.float32).reshape(C, 1)
    )
    wvT = np.ascontiguousarray(np.asarray(Wv, np.float32).T.astype(bf))
    kv_b = {}
    for b in range(B):
        kvx = kv[:, b].reshape(T, C, NQ).transpose(1, 0, 2).reshape(C, M)
        kvxT = kvx.T.reshape(NCH, 128, C).transpose(1, 0, 2).reshape(128, M)
        kv_b[b] = (
            np.ascontiguousarray(kvx.astype(bf)),
            np.ascontiguousarray(kvxT.astype(p_np)),
        )
    in_maps = []
    for core in range(N_CORES):
        b, t = divmod(core, T)
        qx = np.ascontiguousarray(q[t, b].reshape(C, NQ).astype(bf))
        in_maps.append({
            "qx": qx, "aT": aT, "bqk": bqk, "wvT": wvT,
            "kvx": kv_b[b][0], "kvxT": kv_b[b][1],
        })
    return in_maps


def _assemble(results, bv):
    full = np.empty((B, T * NQ, C), np.float32)
    for core in range(N_CORES):
        b, t = divmod(core, T)
        o2 = results[core]["out2"]
        oT = o2[:, :NQ]                              # [C, NQ] unnormalized
        rsv = o2[:, NQ:]                             # [p, chain*8+nb] partial sums
        nch = rsv.shape[1] // (NQ // 128)
        r = sum(rsv[:, ci * 8:(ci + 1) * 8] for ci in range(nch))
        r = r.T.reshape(NQ)                          # [p, nb] = rowsum(nb*128+p)
        full[b, t * NQ:(t + 1) * NQ] = (oT / r[None, :]).T
    full += np.asarray(bv, np.float32)[None, None, :]
    return full


def kernel(query, key_value, Wq, bq, Wk, bk, Wv, bv, **run_kwargs):
    global _NC
    if _NC is None:
        _NC = build_nc()
    in_maps = _prepare_in_maps(query, key_value, Wq, bq, Wk, bk, Wv, bv)
    res = run_bass_kernel_spmd(_NC, in_maps, list(range(N_CORES)), **run_kwargs)
    out = _assemble(res.results, bv)
    if run_kwargs:
        return out, res
    return out


# revision 20
# speedup vs baseline: 1.2531x; 1.0034x over previous
"""Fused cross-attention kernel for Trainium2 (Bass/Tile), 8-core SPMD.

Problem: query/key_value [T=4, B=2, C=128, H=32, W=32] -> tokens [B, N=4096, C],
QKV projections (128x128), full softmax attention over N tokens per batch.

Sharding: core = b*4 + t handles batch b, query tokens [t*1024, (t+1)*1024)
against all 4096 K/V tokens of batch b.

Algebraic restructure (vs. materializing Q/K/V):
  scores:  S^T[m,n] = x_kv[m] . qk[n]   with  qk = (Wk^T Wq) x_q + Wk^T bq
           (A = Wk^T Wq precomputed on host; bk shifts all scores of a row
           equally and drops out of softmax exactly)
  output:  O^T = Wv Z / rowsum,  Z[c,n] = sum_m x_kv[m,c] P[m,n]
           (V-projection pulled out of the attention sum by linearity)
So the device only runs: one 128x128 projection (qk), the two big
attention matmuls (S and Z), one final 128x128 matmul (Wv Z), exp, and
16-bit rowsum accumulation. No K/V projection matmuls, no per-chunk
PSUM->SBUF projection copies.

Per m-chunk (128 kv tokens, 32 chunks):
  pss [m=128, n=1024] = kvx_chunk^T @ qk       (PE, bf16, 2x512-col matmuls)
  p   = exp(SCALE * pss)                       (ACT, PSUM->SBUF, 16-bit out)
  psz [c=128, n=1024] += kvxT_chunk^T @ p      (PE, accumulated over chunks)
  acc_i += p                                   (DVE 2-byte 2x-mode adds)
Rowsums land pre-transposed via tiny PE matmuls acc_i^T @ ones accumulated
in PSUM [n-part, nb]; normalization and the final [C,NQ]->[NQ,C] transpose
happen on host (host already assembles shards and adds bv).

ldweights-only filler instructions (no PSUM write, no semaphores) can be
interleaved to keep the PE busy streak alive for the HAM clock ramp.

Inputs prepacked bf16 on host; P is 16-bit (exp <= e^7.7 ~ 2200 fits both
f16/bf16; validated ~3.6e-3 rel err end-to-end vs the 2e-2 gate).
"""

import math
from contextlib import ExitStack

import numpy as np
import ml_dtypes

import concourse.bass as bass
import concourse.mybir as mybir
import concourse.tile as tile
from concourse import bacc
from concourse.bass_utils import run_bass_kernel_spmd

F32 = mybir.dt.float32
F32R = mybir.dt.float32r
BF16 = mybir.dt.bfloat16
F16 = mybir.dt.float16
AF = mybir.ActivationFunctionType

C = 128        # model dim
NQ = 1024      # query tokens per core
M = 4096       # kv tokens per batch
NCH = M // 128 # m chunks
T = 4
B = 2
SCALE = 1.0 / math.sqrt(float(C))
N_CORES = 8

CFG = dict(
    p_dtype="f16",  # "f16" | "bf16" | "f32r": exp output / kvxT / rowsum dtype
    p_bufs=8,       # exp output SBUF buffers
    ps_s_bufs=3,    # score PSUM buffers ([128,1024] = 2 banks each)
    pe_warm=24,     # ldweights warm-ups during the DMA window
    fillers=2,      # ldweights fillers per chunk (hold the PE HAM streak)
    head_fill=8,    # ldweights fillers between qk proj and chunk 0
    z_on_act=True,  # drain Z psum->sbuf on ACT (idle after last exp)
    fillers_dep=True,  # fillers read p (un-hoistable, interleave per chunk)
    unroll=8,       # kernel bodies per For_i iteration (amortizes barrier)
)

_P_DT = {"f16": F16, "bf16": BF16, "f32r": F32R}
_P_NP = {"f16": np.float16, "bf16": ml_dtypes.bfloat16, "f32r": np.float32}
_N_CHAINS = {"f16": 2, "bf16": 4, "f32r": 2}

_NC = None


def build_nc(reps=1, loop_reps=0, **overrides):
    cfg = dict(CFG)
    cfg.update(overrides)
    p_dt = _P_DT[cfg["p_dtype"]]
    acc_dt = F32 if cfg["p_dtype"] == "f32r" else p_dt
    n_chains = _N_CHAINS[cfg["p_dtype"]]

    nc = bacc.Bacc()
    qx = nc.dram_tensor("qx", [C, NQ], BF16, kind="ExternalInput")
    aT = nc.dram_tensor("aT", [C, C], BF16, kind="ExternalInput")
    bqk = nc.dram_tensor("bqk", [C, 1], F32, kind="ExternalInput")
    wvT = nc.dram_tensor("wvT", [C, C], BF16, kind="ExternalInput")
    kvx = nc.dram_tensor("kvx", [C, M], BF16, kind="ExternalInput")
    kvxT = nc.dram_tensor("kvxT", [C, M], p_dt, kind="ExternalInput")
    out2 = nc.dram_tensor("out2", [C, NQ + (NQ // C) * n_chains], F32,
                          kind="ExternalOutput")

    unroll = cfg["unroll"]
    if loop_reps and loop_reps % unroll == 0 and loop_reps >= unroll:
        loop_iters, reps = loop_reps // unroll, reps * unroll
    elif loop_reps:
        loop_iters = loop_reps
    else:
        loop_iters = 0

    with tile.TileContext(nc) as tc, ExitStack() as ctx:
        const = ctx.enter_context(tc.tile_pool(name="const", bufs=1))
        proj = ctx.enter_context(tc.tile_pool(name="proj", bufs=1))
        pwork = ctx.enter_context(tc.tile_pool(name="pwork", bufs=cfg["p_bufs"]))
        owork = ctx.enter_context(tc.tile_pool(name="owork", bufs=2))
        psum = ctx.enter_context(tc.tile_pool(name="psum", bufs=2, space="PSUM"))

        def misc_tile(name):
            # borrow a rotating score-PSUM buffer for small/late matmuls
            return psum.tile([128, NQ], F32, tag="ps_s",
                             bufs=cfg["ps_s_bufs"], name=name)

        # Constants (gpsimd, no DMA deps). Warm the exp table first so the
        # 1.5us table load overlaps the input DMAs / NEFF preamble.
        ones_f32 = const.tile([128, 1], F32)
        nc.gpsimd.memset(ones_f32, 1.0)
        warm = const.tile([128, 1], F32)
        nc.scalar.activation(warm, ones_f32, AF.Exp)
        ones_p = const.tile([128, 1], acc_dt)
        nc.gpsimd.memset(ones_p, 1.0)
        warm_w = const.tile([128, 128], BF16)
        nc.gpsimd.memset(warm_w, 1.0)

        # PE warm-up: ldweights-only ops (no PSUM, no cross-engine deps)
        # to build a continuous-busy streak for the HAM clock ramp.
        for _w in range(cfg["pe_warm"]):
            nc.tensor.ldweights(warm_w)

        # Input DMAs, spread across the SP and ACT HWDGE rings.
        qx_sb = const.tile([C, NQ], BF16)
        nc.sync.dma_start(qx_sb, qx[:])
        aT_sb = const.tile([C, C], BF16)
        nc.sync.dma_start(aT_sb, aT[:])
        bqk_sb = const.tile([C, 1], F32)
        nc.sync.dma_start(bqk_sb, bqk[:])
        wvT_sb = const.tile([C, C], BF16)
        nc.sync.dma_start(wvT_sb, wvT[:])
        kvx_sb = const.tile([C, M], BF16)
        nc.sync.dma_start(kvx_sb[:, 0:1024], kvx[:, 0:1024])
        nc.scalar.dma_start(kvx_sb[:, 1024:2560], kvx[:, 1024:2560])
        nc.scalar.dma_start(kvx_sb[:, 2560:4096], kvx[:, 2560:4096])
        kvxT_sb = const.tile([C, M], p_dt)
        nc.sync.dma_start(kvxT_sb[:, 0:2048], kvxT[:, 0:2048])
        nc.scalar.dma_start(kvxT_sb[:, 2048:4096], kvxT[:, 2048:4096])

        loop_cm = tc.For_i(0, loop_iters, 1) if loop_iters else None
        if loop_cm is not None:
            loop_cm.__enter__()
        for _rep in range(reps):
            # ---- qk projection: qk = A @ qx + bqk ----
            # (matmul outputs may not cross a PSUM bank: 512 f32 cols max)
            psq = misc_tile("psq")
            for h in range(2):
                nc.tensor.matmul(psq[:, h * 512:(h + 1) * 512], lhsT=aT_sb,
                                 rhs=qx_sb[:, h * 512:(h + 1) * 512],
                                 start=True, stop=True)
            qk_sb = proj.tile([C, NQ], BF16, name="qk_sb")
            with nc.allow_low_precision(reason="bf16 qk tokens"):
                # halves on different engines so they run in parallel and
                # the first chunk's exp starts earlier
                nc.vector.tensor_scalar_add(qk_sb[:, 0:512], psq[:, 0:512],
                                            bqk_sb)
                nc.scalar.activation(qk_sb[:, 512:1024], psq[:, 512:1024],
                                     AF.Identity, bias=bqk_sb)
            for _f in range(cfg["head_fill"]):
                nc.tensor.ldweights(warm_w)

            # ---- chunk loop ----
            psz = psum.tile([128, NQ], F32, tag="ps_z", bufs=1, name="psz")
            accs = [owork.tile([128, NQ], acc_dt, tag=f"acc{i}", bufs=1,
                               name=f"acc{i}") for i in range(n_chains)]
            for j in range(NCH):
                pss = psum.tile([128, NQ], F32, tag="ps_s",
                                bufs=cfg["ps_s_bufs"])
                for h in range(2):
                    nc.tensor.matmul(pss[:, h * 512:(h + 1) * 512],
                                     lhsT=kvx_sb[:, j * 128:(j + 1) * 128],
                                     rhs=qk_sb[:, h * 512:(h + 1) * 512],
                                     start=True, stop=True)
                if cfg["fillers"] and not cfg["fillers_dep"]:
                    for _f in range((cfg["fillers"] + 1) // 2):
                        nc.tensor.ldweights(warm_w)
                p = pwork.tile([128, NQ], p_dt, tag="p_sb", bufs=cfg["p_bufs"])
                nc.scalar.activation(p, pss, AF.Exp, scale=SCALE)
                for h in range(2):
                    nc.tensor.matmul(psz[:, h * 512:(h + 1) * 512],
                                     lhsT=kvxT_sb[:, j * 128:(j + 1) * 128],
                                     rhs=p[:, h * 512:(h + 1) * 512],
                                     start=(j == 0), stop=(j == NCH - 1))
                if cfg["fillers"]:
                    nf = (cfg["fillers"] if cfg["fillers_dep"]
                          else cfg["fillers"] // 2)
                    for _f in range(nf):
                        # reading p makes the filler depend on this chunk's
                        # exp, so the scheduler cannot hoist it into a blob
                        nc.tensor.ldweights(p[:, _f * 128:(_f + 1) * 128])
                acc = accs[j % n_chains]
                pv = p.bitcast(F32) if cfg["p_dtype"] == "f32r" else p
                with nc.allow_low_precision(reason="16-bit rowsum chains"):
                    if j < n_chains:
                        nc.vector.tensor_copy(acc, pv)
                    else:
                        nc.vector.tensor_add(acc, acc, pv)

            # ---- rowsums: transpose-reduce acc_i^T @ ones into per-chain
            # psr columns (independent groups; host sums the chains). n lands
            # on partitions for free. Acquired after the loop so the score-
            # PSUM rotation is not starved at chunks 30-31; the matmuls are
            # PE-side and run parallel to the ACT z-drain.
            psr = misc_tile("psr")
            for ci in range(n_chains):
                for nb in range(NQ // 128):
                    nc.tensor.matmul(
                        psr[:, ci * 8 + nb:ci * 8 + nb + 1],
                        lhsT=accs[ci][:, nb * 128:(nb + 1) * 128],
                        rhs=ones_p, start=True, stop=True)

            # ---- O^T = Wv @ Z (unnormalized; host divides by rowsums),
            # quarter-split so drain/matmul/copy/DMA pipeline across engines.
            # Rowsums ride along in cols [NQ, NQ+8) of the same output. ----
            z_sb = proj.tile([C, NQ], BF16, name="z_sb")
            pso = misc_tile("pso")
            nrs = (NQ // 128) * n_chains
            o_sb = owork.tile([128, NQ + nrs], F32, tag="o_sb", bufs=2,
                              name="o_sb")
            nc.vector.tensor_copy(o_sb[:, NQ:NQ + nrs], psr[:, 0:nrs])
            for q in range(4):
                qs = slice(q * 256, (q + 1) * 256)
                with nc.allow_low_precision(reason="bf16 z"):
                    if cfg["z_on_act"]:
                        nc.scalar.copy(z_sb[:, qs], psz[:, qs])
                    else:
                        nc.vector.tensor_copy(z_sb[:, qs], psz[:, qs])
                nc.tensor.matmul(pso[:, qs], lhsT=wvT_sb, rhs=z_sb[:, qs],
                                 start=True, stop=True)
                nc.vector.tensor_copy(o_sb[:, qs], pso[:, qs])
                if q < 3:
                    nc.sync.dma_start(out2[:, qs], o_sb[:, qs])
                else:
                    nc.sync.dma_start(out2[:, 768:NQ + nrs],
                                      o_sb[:, 768:NQ + nrs])
        if loop_cm is not None:
            loop_cm.__exit__(None, None, None)
    nc.compile()
    return nc


def _prepare_in_maps(query, key_value, Wq, bq, Wk, bk, Wv, bv, p_dtype=None):
    bf = ml_dtypes.bfloat16
    p_np = _P_NP[p_dtype or CFG["p_dtype"]]
    q = np.asarray(query, np.float32)
    kv = np.asarray(key_value, np.float32)
    Wq64 = np.asarray(Wq, np.float64)
    Wk64 = np.asarray(Wk, np.float64)
    aT = np.ascontiguousarray((Wq64.T @ Wk64).astype(bf))
    bqk = np.ascontiguousarray(
        (Wk64.T @ np.asarray(bq, np.float64)).astype(np.float32).reshape(C, 1)
    )
    wvT = np.ascontiguousarray(np.asarray(Wv, np.float32).T.astype(bf))
    kv_b = {}
    for b in range(B):
        kvx = kv[:, b].reshape(T, C, NQ).transpose(1, 0, 2).reshape(C, M)
        kvxT = kvx.T.reshape(NCH, 128, C).transpose(1, 0, 2).reshape(128, M)
        kv_b[b] = (
            np.ascontiguousarray(kvx.astype(bf)),
            np.ascontiguousarray(kvxT.astype(p_np)),
        )
    in_maps = []
    for core in range(N_CORES):
        b, t = divmod(core, T)
        qx = np.ascontiguousarray(q[t, b].reshape(C, NQ).astype(bf))
        in_maps.append({
            "qx": qx, "aT": aT, "bqk": bqk, "wvT": wvT,
            "kvx": kv_b[b][0], "kvxT": kv_b[b][1],
        })
    return in_maps


def _assemble(results, bv):
    full = np.empty((B, T * NQ, C), np.float32)
    for core in range(N_CORES):
        b, t = divmod(core, T)
        o2 = results[core]["out2"]
        oT = o2[:, :NQ]                              # [C, NQ] unnormalized
        rsv = o2[:, NQ:]                             # [p, chain*8+nb] partial sums
        nch = rsv.shape[1] // (NQ // 128)
        r = sum(rsv[:, ci * 8:(ci + 1) * 8] for ci in range(nch))
        r = r.T.reshape(NQ)                          # [p, nb] = rowsum(nb*128+p)
        full[b, t * NQ:(t + 1) * NQ] = (oT / r[None, :]).T
    full += np.asarray(bv, np.float32)[None, None, :]
    return full


def kernel(query, key_value, Wq, bq, Wk, bk, Wv, bv, **run_kwargs):
    global _NC
    if _NC is None:
        _NC = build_nc()
    in_maps = _prepare_in_maps(query, key_value, Wq, bq, Wk, bk, Wv, bv)
    res = run_bass_kernel_spmd(_NC, in_maps, list(range(N_CORES)), **run_kwargs)
    out = _assemble(res.results, bv)
    if run_kwargs:
        return out, res
    return out


# revision 21
# speedup vs baseline: 1.7782x; 1.4190x over previous
"""Fused cross-attention kernel for Trainium2 (Bass/Tile), 8-core SPMD.

Problem: query/key_value [T=4, B=2, C=128, H=32, W=32] -> tokens [B, N=4096, C],
QKV projections (128x128), full softmax attention over N tokens per batch.

Sharding: core = b*4 + t handles batch b, query tokens [t*1024, (t+1)*1024)
against all 4096 K/V tokens of batch b.

Algebraic restructure (vs. materializing Q/K/V):
  scores:  S^T[m,n] = x_kv[m] . qk[n]   with  qk = (Wk^T Wq) x_q + Wk^T bq
           (A = Wk^T Wq precomputed on host; bk shifts all scores of a row
           equally and drops out of softmax exactly)
  output:  O^T = Wv Z / rowsum,  Z[c,n] = sum_m x_kv[m,c] P[m,n]
           (V-projection pulled out of the attention sum by linearity)
So the device only runs: one 128x128 projection (qk), the two big
attention matmuls (S and Z), one final 128x128 matmul (Wv Z), exp, and
16-bit rowsum accumulation. No K/V projection matmuls, no per-chunk
PSUM->SBUF projection copies.

Per m-chunk (128 kv tokens, 32 chunks):
  pss [m=128, n=1024] = kvx_chunk^T @ qk       (PE, bf16, 2x512-col matmuls)
  p   = exp(SCALE * pss)                       (ACT, PSUM->SBUF, 16-bit out)
  psz [c=128, n=1024] += kvxT_chunk^T @ p      (PE, accumulated over chunks)
  acc_i += p                                   (DVE 2-byte 2x-mode adds)
Rowsums land pre-transposed via tiny PE matmuls acc_i^T @ ones accumulated
in PSUM [n-part, nb]; normalization and the final [C,NQ]->[NQ,C] transpose
happen on host (host already assembles shards and adds bv).

ldweights-only filler instructions (no PSUM write, no semaphores) can be
interleaved to keep the PE busy streak alive for the HAM clock ramp.

Inputs prepacked bf16 on host; P is 16-bit (exp <= e^7.7 ~ 2200 fits both
f16/bf16; validated ~3.6e-3 rel err end-to-end vs the 2e-2 gate).
"""

import math
from contextlib import ExitStack

import numpy as np
import ml_dtypes

import concourse.bass as bass
import concourse.mybir as mybir
import concourse.tile as tile
from concourse import bacc
from concourse.bass_utils import run_bass_kernel_spmd

F32 = mybir.dt.float32
F32R = mybir.dt.float32r
BF16 = mybir.dt.bfloat16
F16 = mybir.dt.float16
AF = mybir.ActivationFunctionType

C = 128        # model dim
NQ = 1024      # query tokens per core
M = 4096       # kv tokens per batch
NCH = M // 128 # m chunks
T = 4
B = 2
SCALE = 1.0 / math.sqrt(float(C))
N_CORES = 8

CFG = dict(
    p_dtype="f16",  # "f16" | "bf16" | "f32r": exp output / kvxT / rowsum dtype
    p_bufs=8,       # exp output SBUF buffers
    ps_s_bufs=3,    # score PSUM buffers ([128,1024] = 2 banks each)
    pe_warm=24,     # ldweights warm-ups during the DMA window
    fillers=1,      # ldweights fillers per chunk (hold the PE HAM streak)
    head_fill=8,    # ldweights fillers between qk proj and chunk 0
    z_on_act=True,  # drain Z psum->sbuf on ACT (idle after last exp)
    fillers_dep=True,  # fillers read p (un-hoistable, interleave per chunk)
    unroll=8,       # kernel bodies per For_i iteration (amortizes barrier)
)

_P_DT = {"f16": F16, "bf16": BF16, "f32r": F32R}
_P_NP = {"f16": np.float16, "bf16": ml_dtypes.bfloat16, "f32r": np.float32}
_N_CHAINS = {"f16": 2, "bf16": 4, "f32r": 2}

_NC = None


def build_nc(reps=1, loop_reps=0, **overrides):
    cfg = dict(CFG)
    cfg.update(overrides)
    p_dt = _P_DT[cfg["p_dtype"]]
    acc_dt = F32 if cfg["p_dtype"] == "f32r" else p_dt
    n_chains = _N_CHAINS[cfg["p_dtype"]]

    nc = bacc.Bacc()
    qx = nc.dram_tensor("qx", [C, NQ], BF16, kind="ExternalInput")
    aT = nc.dram_tensor("aT", [C, C], BF16, kind="ExternalInput")
    bqk = nc.dram_tensor("bqk", [C, 1], F32, kind="ExternalInput")
    wvT = nc.dram_tensor("wvT", [C, C], BF16, kind="ExternalInput")
    kvx = nc.dram_tensor("kvx", [C, M], BF16, kind="ExternalInput")
    kvxT = nc.dram_tensor("kvxT", [C, M], p_dt, kind="ExternalInput")
    out2 = nc.dram_tensor("out2", [C, NQ + (NQ // C) * n_chains], F32,
                          kind="ExternalOutput")

    unroll = cfg["unroll"]
    if loop_reps and loop_reps % unroll == 0 and loop_reps >= unroll:
        loop_iters, reps = loop_reps // unroll, reps * unroll
    elif loop_reps:
        loop_iters = loop_reps
    else:
        loop_iters = 0

    with tile.TileContext(nc) as tc, ExitStack() as ctx:
        const = ctx.enter_context(tc.tile_pool(name="const", bufs=1))
        proj = ctx.enter_context(tc.tile_pool(name="proj", bufs=1))
        pwork = ctx.enter_context(tc.tile_pool(name="pwork", bufs=cfg["p_bufs"]))
        owork = ctx.enter_context(tc.tile_pool(name="owork", bufs=2))
        psum = ctx.enter_context(tc.tile_pool(name="psum", bufs=2, space="PSUM"))

        def misc_tile(name):
            # borrow a rotating score-PSUM buffer for small/late matmuls
            return psum.tile([128, NQ], F32, tag="ps_s",
                             bufs=cfg["ps_s_bufs"], name=name)

        # Constants (gpsimd, no DMA deps). Warm the exp table first so the
        # 1.5us table load overlaps the input DMAs / NEFF preamble.
        ones_f32 = const.tile([128, 1], F32)
        nc.gpsimd.memset(ones_f32, 1.0)
        warm = const.tile([128, 1], F32)
        nc.scalar.activation(warm, ones_f32, AF.Exp)
        ones_p = const.tile([128, 1], acc_dt)
        nc.gpsimd.memset(ones_p, 1.0)
        warm_w = const.tile([128, 128], BF16)
        nc.gpsimd.memset(warm_w, 1.0)

        # PE warm-up: ldweights-only ops (no PSUM, no cross-engine deps)
        # to build a continuous-busy streak for the HAM clock ramp.
        for _w in range(cfg["pe_warm"]):
            nc.tensor.ldweights(warm_w)

        # Input DMAs, spread across the SP and ACT HWDGE rings.
        qx_sb = const.tile([C, NQ], BF16)
        nc.sync.dma_start(qx_sb, qx[:])
        aT_sb = const.tile([C, C], BF16)
        nc.sync.dma_start(aT_sb, aT[:])
        bqk_sb = const.tile([C, 1], F32)
        nc.sync.dma_start(bqk_sb, bqk[:])
        wvT_sb = const.tile([C, C], BF16)
        nc.sync.dma_start(wvT_sb, wvT[:])
        kvx_sb = const.tile([C, M], BF16)
        nc.sync.dma_start(kvx_sb[:, 0:1024], kvx[:, 0:1024])
        nc.scalar.dma_start(kvx_sb[:, 1024:2560], kvx[:, 1024:2560])
        nc.scalar.dma_start(kvx_sb[:, 2560:4096], kvx[:, 2560:4096])
        kvxT_sb = const.tile([C, M], p_dt)
        nc.sync.dma_start(kvxT_sb[:, 0:2048], kvxT[:, 0:2048])
        nc.scalar.dma_start(kvxT_sb[:, 2048:4096], kvxT[:, 2048:4096])

        loop_cm = tc.For_i(0, loop_iters, 1) if loop_iters else None
        if loop_cm is not None:
            loop_cm.__enter__()
        for _rep in range(reps):
            # ---- qk projection: qk = A @ qx + bqk ----
            # (matmul outputs may not cross a PSUM bank: 512 f32 cols max)
            psq = misc_tile("psq")
            for h in range(2):
                nc.tensor.matmul(psq[:, h * 512:(h + 1) * 512], lhsT=aT_sb,
                                 rhs=qx_sb[:, h * 512:(h + 1) * 512],
                                 start=True, stop=True)
            qk_sb = proj.tile([C, NQ], BF16, name="qk_sb")
            with nc.allow_low_precision(reason="bf16 qk tokens"):
                # halves on different engines so they run in parallel and
                # the first chunk's exp starts earlier
                nc.vector.tensor_scalar_add(qk_sb[:, 0:512], psq[:, 0:512],
                                            bqk_sb)
                nc.scalar.activation(qk_sb[:, 512:1024], psq[:, 512:1024],
                                     AF.Identity, bias=bqk_sb)
            for _f in range(cfg["head_fill"]):
                nc.tensor.ldweights(warm_w)

            # ---- chunk loop ----
            psz = psum.tile([128, NQ], F32, tag="ps_z", bufs=1, name="psz")
            accs = [owork.tile([128, NQ], acc_dt, tag=f"acc{i}", bufs=1,
                               name=f"acc{i}") for i in range(n_chains)]
            for j in range(NCH):
                pss = psum.tile([128, NQ], F32, tag="ps_s",
                                bufs=cfg["ps_s_bufs"])
                for h in range(2):
                    nc.tensor.matmul(pss[:, h * 512:(h + 1) * 512],
                                     lhsT=kvx_sb[:, j * 128:(j + 1) * 128],
                                     rhs=qk_sb[:, h * 512:(h + 1) * 512],
                                     start=True, stop=True)
                if cfg["fillers"] and not cfg["fillers_dep"]:
                    for _f in range((cfg["fillers"] + 1) // 2):
                        nc.tensor.ldweights(warm_w)
                p = pwork.tile([128, NQ], p_dt, tag="p_sb", bufs=cfg["p_bufs"])
                nc.scalar.activation(p, pss, AF.Exp, scale=SCALE)
                for h in range(2):
                    nc.tensor.matmul(psz[:, h * 512:(h + 1) * 512],
                                     lhsT=kvxT_sb[:, j * 128:(j + 1) * 128],
                                     rhs=p[:, h * 512:(h + 1) * 512],
                                     start=(j == 0), stop=(j == NCH - 1))
                if cfg["fillers"]:
                    nf = (cfg["fillers"] if cfg["fillers_dep"]
                          else cfg["fillers"] // 2)
                    for _f in range(nf):
                        # reading p makes the filler depend on this chunk's
                        # exp, so the scheduler cannot hoist it into a blob
                        nc.tensor.ldweights(p[:, _f * 128:(_f + 1) * 128])
                acc = accs[j % n_chains]
                pv = p.bitcast(F32) if cfg["p_dtype"] == "f32r" else p
                with nc.allow_low_precision(reason="16-bit rowsum chains"):
                    if j < n_chains:
                        nc.vector.tensor_copy(acc, pv)
                    else:
                        nc.vector.tensor_add(acc, acc, pv)

            # ---- rowsums: transpose-reduce acc_i^T @ ones into per-chain
            # psr columns (independent groups; host sums the chains). n lands
            # on partitions for free. Acquired after the loop so the score-
            # PSUM rotation is not starved at chunks 30-31; the matmuls are
            # PE-side and run parallel to the ACT z-drain.
            psr = misc_tile("psr")
            for ci in range(n_chains):
                for nb in range(NQ // 128):
                    nc.tensor.matmul(
                        psr[:, ci * 8 + nb:ci * 8 + nb + 1],
                        lhsT=accs[ci][:, nb * 128:(nb + 1) * 128],
                        rhs=ones_p, start=True, stop=True)

            # ---- O^T = Wv @ Z (unnormalized; host divides by rowsums),
            # quarter-split so drain/matmul/copy/DMA pipeline across engines.
            # Rowsums ride along in cols [NQ, NQ+8) of the same output. ----
            z_sb = proj.tile([C, NQ], BF16, name="z_sb")
            pso = misc_tile("pso")
            nrs = (NQ // 128) * n_chains
            o_sb = owork.tile([128, NQ + nrs], F32, tag="o_sb", bufs=2,
                              name="o_sb")
            nc.vector.tensor_copy(o_sb[:, NQ:NQ + nrs], psr[:, 0:nrs])
            for q in range(4):
                qs = slice(q * 256, (q + 1) * 256)
                with nc.allow_low_precision(reason="bf16 z"):
                    if cfg["z_on_act"]:
                        nc.scalar.copy(z_sb[:, qs], psz[:, qs])
                    else:
                        nc.vector.tensor_copy(z_sb[:, qs], psz[:, qs])
                nc.tensor.matmul(pso[:, qs], lhsT=wvT_sb, rhs=z_sb[:, qs],
                                 start=True, stop=True)
                nc.vector.tensor_copy(o_sb[:, qs], pso[:, qs])
                if q < 3:
                    nc.sync.dma_start(out2[:, qs], o_sb[:, qs])
                else:
                    nc.sync.dma_start(out2[:, 768:NQ + nrs],
                                      o_sb[:, 768:NQ + nrs])
        if loop_cm is not None:
            loop_cm.__exit__(None, None, None)
    nc.compile()
    return nc


def _prepare_in_maps(query, key_value, Wq, bq, Wk, bk, Wv, bv, p_dtype=None):
    bf = ml_dtypes.bfloat16
    p_np = _P_NP[p_dtype or CFG["p_dtype"]]
    q = np.asarray(query, np.float32)
    kv = np.asarray(key_value, np.float32)
    Wq64 = np.asarray(Wq, np.float64)
    Wk64 = np.asarray(Wk, np.float64)
    aT = np.ascontiguousarray((Wq64.T @ Wk64).astype(bf))
    bqk = np.ascontiguousarray(
        (Wk64.T @ np.asarray(bq, np.float64)).astype(np.float32).reshape(C, 1)
    )
    wvT = np.ascontiguousarray(np.asarray(Wv, np.float32).T.astype(bf))
    kv_b = {}
    for b in range(B):
        kvx = kv[:, b].reshape(T, C, NQ).transpose(1, 0, 2).reshape(C, M)
        kvxT = kvx.T.reshape(NCH, 128, C).transpose(1, 0, 2).reshape(128, M)
        kv_b[b] = (
            np.ascontiguousarray(kvx.astype(bf)),
            np.ascontiguousarray(kvxT.astype(p_np)),
        )
    in_maps = []
    for core in range(N_CORES):
        b, t = divmod(core, T)
        qx = np.ascontiguousarray(q[t, b].reshape(C, NQ).astype(bf))
        in_maps.append({
            "qx": qx, "aT": aT, "bqk": bqk, "wvT": wvT,
            "kvx": kv_b[b][0], "kvxT": kv_b[b][1],
        })
    return in_maps


def _assemble(results, bv):
    full = np.empty((B, T * NQ, C), np.float32)
    for core in range(N_CORES):
        b, t = divmod(core, T)
        o2 = results[core]["out2"]
        oT = o2[:, :NQ]                              # [C, NQ] unnormalized
        rsv = o2[:, NQ:]                             # [p, chain*8+nb] partial sums
        nch = rsv.shape[1] // (NQ // 128)
        r = sum(rsv[:, ci * 8:(ci + 1) * 8] for ci in range(nch))
        r = r.T.reshape(NQ)                          # [p, nb] = rowsum(nb*128+p)
        full[b, t * NQ:(t + 1) * NQ] = (oT / r[None, :]).T
    full += np.asarray(bv, np.float32)[None, None, :]
    return full


def kernel(query, key_value, Wq, bq, Wk, bk, Wv, bv, **run_kwargs):
    global _NC
    if _NC is None:
        _NC = build_nc()
    in_maps = _prepare_in_maps(query, key_value, Wq, bq, Wk, bk, Wv, bv)
    res = run_bass_kernel_spmd(_NC, in_maps, list(range(N_CORES)), **run_kwargs)
    out = _assemble(res.results, bv)
    if run_kwargs:
        return out, res
    return out
